# revision 18
# baseline (speedup 1.0000x reference)
"""Bass/Trainium2 kernel for nn_BlockGNN (2-layer GATv2 + MLP) on 8 NeuronCores.

Strategy (per spec sharding hint):
  - Nodes partitioned across 8 cores by destination (6250 nodes/core).
  - Edges routed to the core owning their dst; self-loops appended; packed
    into 49 windows of 128 dst-nodes, each window padded to a uniform KT
    tiles of 128 edge slots (SPMD: same program on all cores).
  - Host performs the halo gather: for every edge slot it gathers the
    source node's raw features into a feature-major slab (x[src].T), which
    is uploaded per layer. One-hot Sel/SelT matrices (fp8) encode each
    slot's destination within its window.
  - Per layer (one SPMD launch per layer; host re-shards between layers),
    per 128-edge tile, on device:
      PE builds z = ea@We + (xr[dst] + bl + br) + xg@Wl in PSUM
        (SelT one-hot matmul expands xr; the slab is the matmul lhsT so
        the Wl projection happens on the tensor engine),
      leaky_relu folds in as z + 0.8*relu(-z) (ACT relu + scaled-identity
        matmul), attention logits via DVE mul+reduce, exp on ACT,
      a second PE pass computes the value projection xl = xg@Wl + bl,
      and one segment-reduction matmul per tile accumulates
        [sum ex*xl | sum ex] per dst node into PSUM.
  - Node stage per window: divide by denominators, +bias, PE transpose,
    2-matmul MLP, write the local slice of the next layer's features.
"""

import os
import sys
import time

import numpy as np

os.environ.setdefault("MYCRO_LOCAL_CACHE", "1")

for _p in ("/opt/trn_rl_repo",):
    if os.path.isdir(_p) and _p not in sys.path:
        sys.path.append(_p)

import concourse.bass as bass
import concourse.bacc as bacc
import concourse.mybir as mybir
import concourse.tile as tile
from concourse.bass import AP
from concourse.bass_utils import run_bass_kernel_spmd

F32 = mybir.dt.float32
BF16 = mybir.dt.bfloat16
FP8 = mybir.dt.float8e4
I32 = mybir.dt.int32

NPF32 = np.float32
NPBF16 = mybir.dt.np(BF16)
NPFP8 = mybir.dt.np(FP8)

# Problem constants
N, E, D, H, CDIM, L = 50000, 800000, 128, 4, 32, 2
P = 128
NCORES = 8
NEG = 0.2

# dtype knobs
_EDT = os.environ.get("KDT_EDGE", "bf16")
_SDT = os.environ.get("KDT_SEL", "fp8")
EDGE_DT = {"bf16": BF16, "f32": F32}[_EDT]
NP_EDGE = mybir.dt.np(EDGE_DT)
SEL_DT = {"fp8": FP8, "bf16": BF16, "f32": F32}[_SDT]
NP_SEL = mybir.dt.np(SEL_DT)
TRACE = bool(int(os.environ.get("KTRACE", "0")))

LAST_EXEC_NS = []     # per-launch exec_time_ns when tracing
LAST_RESULTS = []     # per-launch BassKernelResults when tracing


def _install_ntff_hook():
    """Provide antenv.axon_hooks (NTFF profiling via the axon PJRT .so)
    when the image lacks it, so run_bass_kernel_spmd(trace=True) works."""
    try:
        import antenv.axon_hooks  # noqa: F401
        return
    except ImportError:
        pass
    import contextlib
    import ctypes
    import types

    try:
        import antenv
    except ImportError:
        return
    so_path = "/opt/axon/libaxon_pjrt.so"
    if not os.path.exists(so_path):
        return
    lib = ctypes.CDLL(so_path)
    if not hasattr(lib, "axon_start_nrt_profile"):
        return
    lib.axon_start_nrt_profile.argtypes = [
        ctypes.POINTER(ctypes.c_int64),
        ctypes.c_size_t,
    ]
    lib.axon_start_nrt_profile.restype = ctypes.c_int64
    lib.axon_stop_nrt_profile.argtypes = [ctypes.c_char_p]
    lib.axon_stop_nrt_profile.restype = ctypes.c_int64

    @contextlib.contextmanager
    def _hook(output_dir, device_ids):
        import jax

        jax.devices()
        if device_ids:
            ids = (ctypes.c_int64 * len(device_ids))(*device_ids)
            rc = lib.axon_start_nrt_profile(ids, len(device_ids))
        else:
            rc = lib.axon_start_nrt_profile(None, 0)
        if rc != 0:
            raise RuntimeError(f"axon_start_nrt_profile rc={rc}")
        try:
            yield
        finally:
            n = lib.axon_stop_nrt_profile(str(output_dir).encode())
            print(f"ntff profile: {n} file(s) -> {output_dir}", file=sys.stderr)

    mod = types.ModuleType("antenv.axon_hooks")
    _state = {"hook": _hook}
    mod.get_axon_ntff_profile_hook = lambda: _state["hook"]
    mod.set_axon_ntff_profile_hook = lambda h: _state.update(hook=h)
    sys.modules["antenv.axon_hooks"] = mod
    antenv.axon_hooks = mod


if TRACE:
    _install_ntff_hook()


def _bcast_last(ap: AP, n: int) -> AP:
    """Append a stride-0 trailing dim of size n to an AP."""
    return AP(ap.tensor, ap.offset, [list(p) for p in ap.ap] + [[0, n]])


def build_layer_nc(cfg, enable_asserts=False):
    """Build the single-layer SPMD program. cfg: nwin, kt (tiles/window)."""
    NWIN, KT = cfg["nwin"], cfg["kt"]
    NLOCP = NWIN * P
    ESLOT = NWIN * KT * P
    assert KT % 2 == 0
    MACROS = []
    j0 = 0
    while j0 < KT:
        wdt = 4 if KT - j0 >= 4 else KT - j0
        MACROS.append((j0, wdt))
        j0 += wdt

    nc = bacc.Bacc(
        "TRN2",
        target_bir_lowering=False,
        debug=False,
        enable_asserts=enable_asserts,
        num_devices=cfg.get("ncores", NCORES),
    )

    # ---- inputs ----
    xgT = nc.dram_tensor("xgT", [P, ESLOT], EDGE_DT, kind="ExternalInput").ap()
    xTloc = nc.dram_tensor("xTloc", [P, NLOCP], F32, kind="ExternalInput").ap()
    Wl_b = nc.dram_tensor("Wl_b", [P, P], EDGE_DT, kind="ExternalInput").ap()
    Wr = nc.dram_tensor("Wr", [P, P], F32, kind="ExternalInput").ap()
    w1 = nc.dram_tensor("w1", [P, P], F32, kind="ExternalInput").ap()
    w2 = nc.dram_tensor("w2", [P, P], F32, kind="ExternalInput").ap()
    We_b = nc.dram_tensor("We_b", [CDIM, P], EDGE_DT, kind="ExternalInput").ap()
    blv = nc.dram_tensor("blv", [1, P], F32, kind="ExternalInput").ap()
    brv = nc.dram_tensor("brv", [1, P], F32, kind="ExternalInput").ap()
    ones1 = nc.dram_tensor("ones1", [1, P], F32, kind="ExternalInput").ap()
    attb4 = nc.dram_tensor("attb4", [P, 4 * P], EDGE_DT, kind="ExternalInput").ap()
    i08b = nc.dram_tensor("i08b", [P, P], EDGE_DT, kind="ExternalInput").ap()
    i128f = nc.dram_tensor("i128f", [P, P], F32, kind="ExternalInput").ap()
    b1c = nc.dram_tensor("b1c", [P, 1], F32, kind="ExternalInput").ap()
    b2c = nc.dram_tensor("b2c", [P, 1], F32, kind="ExternalInput").ap()
    bgc = nc.dram_tensor("bgc", [P, 1], F32, kind="ExternalInput").ap()
    eaT = nc.dram_tensor("eaT", [CDIM, ESLOT], EDGE_DT, kind="ExternalInput").ap()
    seld = nc.dram_tensor("seld", [NWIN, P, KT * P], SEL_DT, kind="ExternalInput").ap()
    selTd = nc.dram_tensor("selTd", [NWIN, P, KT * P], SEL_DT, kind="ExternalInput").ap()
    xoutT = nc.dram_tensor("xoutT", [P, NLOCP], F32, kind="ExternalOutput").ap()

    AF = mybir.ActivationFunctionType
    OP = mybir.AluOpType

    with tile.TileContext(nc) as tc:
        with (
            tc.tile_pool(name="const", bufs=1) as cpool,
            tc.tile_pool(name="win", bufs=2) as wpool,
            tc.tile_pool(name="edge", bufs=3) as epool,
            tc.tile_pool(name="psZ", bufs=2, space="PSUM") as psZ,
            tc.tile_pool(name="psV", bufs=2, space="PSUM") as psV,
            tc.tile_pool(name="psO", bufs=1, space="PSUM") as psO,
            tc.tile_pool(name="psE", bufs=1, space="PSUM") as psE,
        ):
            # ---- load constants/weights to SBUF ----
            def cload(ap, shape, dt, tag):
                t = cpool.tile(shape, dt, tag=tag)
                nc.sync.dma_start(out=t[:], in_=ap)
                return t

            Wlb_s = cload(Wl_b, [P, P], EDGE_DT, tag="Wlb_s")
            Wr_s = cload(Wr, [P, P], F32, tag="Wr_s")
            w1_s = cload(w1, [P, P], F32, tag="w1_s")
            w2_s = cload(w2, [P, P], F32, tag="w2_s")
            We_s = cload(We_b, [CDIM, P], EDGE_DT, tag="We_s")
            blv_s = cload(blv, [1, P], F32, tag="blv_s")
            brv_s = cload(brv, [1, P], F32, tag="brv_s")
            ones_s = cload(ones1, [1, P], F32, tag="ones_s")
            attb_s = cload(attb4, [P, 4 * P], EDGE_DT, tag="attb_s")
            i08b_s = cload(i08b, [P, P], EDGE_DT, tag="i08b_s")
            i128f_s = cload(i128f, [P, P], F32, tag="i128f_s")
            b1c_s = cload(b1c, [P, 1], F32, tag="b1c_s")
            b2c_s = cload(b2c, [P, 1], F32, tag="b2c_s")
            bgc_s = cload(bgc, [P, 1], F32, tag="bgc_s")

            # ---- edge + node stage, per window ----
            for w in range(NWIN):
                xgT_sb = wpool.tile([P, KT * P], EDGE_DT, tag="xgT")
                nc.sync.dma_start(
                    out=xgT_sb[:], in_=xgT[:, w * KT * P : (w + 1) * KT * P]
                )
                eaT_sb = wpool.tile([CDIM, KT * P], EDGE_DT, tag="ea")
                nc.sync.dma_start(
                    out=eaT_sb[:], in_=eaT[:, w * KT * P : (w + 1) * KT * P]
                )
                sel_sb = wpool.tile([P, KT * P], SEL_DT, tag="sel")
                nc.sync.dma_start(out=sel_sb[:], in_=seld[w])
                selT_sb = wpool.tile([P, KT * P], SEL_DT, tag="selT")
                nc.sync.dma_start(out=selT_sb[:], in_=selTd[w])

                # xr for this window's 128 local nodes (+ bl + br folded in)
                xtl_sb = wpool.tile([P, P], F32, tag="xtl")
                nc.sync.dma_start(out=xtl_sb[:], in_=xTloc[:, w * P : (w + 1) * P])
                xr_ps = psE.tile([P, P], F32, tag="xr")
                nc.tensor.matmul(out=xr_ps[:], lhsT=xtl_sb[:], rhs=Wr_s[:],
                                 start=True, stop=False)
                nc.tensor.matmul(out=xr_ps[:], lhsT=ones_s[:], rhs=brv_s[:],
                                 start=False, stop=True)
                xr_sb = wpool.tile([P, P], EDGE_DT, tag="xrs")
                nc.scalar.activation(xr_sb[:], xr_ps[:], AF.Copy)

                out12 = psO.tile([P, 132], F32, tag="o12")

                for mi, (j0, MW) in enumerate(MACROS):
                    zq = psZ.tile([P, MW * P], F32, tag="zq")
                    vq = psV.tile([P, MW * P], F32, tag="vq")
                    # z = ea@We + xr[dst] + xg@Wl  accumulated in psum
                    for u in range(MW):
                        j = j0 + u
                        nc.tensor.matmul(
                            out=zq[:, u * P : (u + 1) * P],
                            lhsT=eaT_sb[:, j * P : (j + 1) * P],
                            rhs=We_s[:],
                            start=(u == 0),
                            stop=False,
                        )
                    for u in range(MW):
                        j = j0 + u
                        nc.tensor.matmul(
                            out=zq[:, u * P : (u + 1) * P],
                            lhsT=selT_sb[:, j * P : (j + 1) * P],
                            rhs=xr_sb[:],
                            start=False,
                            stop=False,
                        )
                    for u in range(MW):
                        j = j0 + u
                        nc.tensor.matmul(
                            out=zq[:, u * P : (u + 1) * P],
                            lhsT=xgT_sb[:, j * P : (j + 1) * P],
                            rhs=Wlb_s[:],
                            start=False,
                            stop=(u == MW - 1),
                        )
                    # value projection xl = xg@Wl + bl (edge-major psum)
                    for u in range(MW):
                        j = j0 + u
                        nc.tensor.matmul(
                            out=vq[:, u * P : (u + 1) * P],
                            lhsT=xgT_sb[:, j * P : (j + 1) * P],
                            rhs=Wlb_s[:],
                            start=(u == 0),
                            stop=False,
                        )
                        nc.tensor.matmul(
                            out=vq[:, u * P : (u + 1) * P],
                            lhsT=ones_s[:],
                            rhs=blv_s[:],
                            start=False,
                            stop=(u == MW - 1),
                        )
                    # leaky: z + 0.8*relu(-z); accumulating matmuls after the
                    # ACT read skip the sim's group bookkeeping (HW just
                    # accumulates into already-written PSUM).
                    mneg = epool.tile([P, MW * P], EDGE_DT, tag="mneg")
                    nc.scalar.activation(mneg[:], zq[:], AF.Relu, scale=-1.0)
                    for u in range(MW):
                        nc.tensor.matmul(
                            out=zq[:, u * P : (u + 1) * P],
                            lhsT=i08b_s[:],
                            rhs=mneg[:, u * P : (u + 1) * P],
                            start=False,
                            stop=False,
                            skip_group_check=True,
                        )
                    # alpha = per-head dot(att, m)
                    am = epool.tile([P, MW * P], EDGE_DT, tag="am")
                    nc.vector.tensor_tensor(am[:], zq[:], attb_s[:, : MW * P],
                                            op=OP.mult)
                    alpha = epool.tile([P, 4 * MW], F32, tag="alpha")
                    nc.vector.tensor_reduce(
                        alpha[:],
                        am[:].rearrange("p (g c) -> p g c", c=CDIM),
                        mybir.AxisListType.X,
                        OP.add,
                    )
                    comb = epool.tile([P, MW * 132], EDGE_DT, tag="comb")
                    comb_v = comb[:].rearrange("p (b f) -> p b f", f=132)
                    nc.scalar.activation(
                        comb_v[:, :, P : P + 4],
                        alpha[:].rearrange("p (b h) -> p b h", h=4),
                        AF.Exp,
                    )
                    nc.vector.tensor_tensor(
                        comb_v[:, :, 0:P].rearrange("p b (h c) -> p b h c", c=CDIM),
                        vq[:].rearrange("p (b h c) -> p b h c", b=MW, c=CDIM),
                        _bcast_last(comb_v[:, :, P : P + 4], CDIM),
                        op=OP.mult,
                    )
                    for u in range(MW):
                        j = j0 + u
                        nc.tensor.matmul(
                            out=out12[:],
                            lhsT=sel_sb[:, j * P : (j + 1) * P],
                            rhs=comb[:, u * 132 : (u + 1) * 132],
                            start=(mi == 0 and u == 0),
                            stop=(mi == len(MACROS) - 1 and u == MW - 1),
                        )

                # ---- window epilogue ----
                de = wpool.tile([P, 4], F32, tag="de")
                nc.vector.tensor_scalar(de[:], out12[:, P : P + 4], 1e-16, None,
                                        OP.add)
                rc = wpool.tile([P, 4], F32, tag="rc")
                nc.vector.reciprocal(rc[:], de[:])
                gat = wpool.tile([P, P], F32, tag="gat")
                for h in range(H):
                    nc.vector.tensor_scalar(
                        gat[:, h * CDIM : (h + 1) * CDIM],
                        out12[:, h * CDIM : (h + 1) * CDIM],
                        rc[:, h : h + 1],
                        None,
                        OP.mult,
                    )
                gatT_ps = psE.tile([P, P], F32, tag="epi")
                nc.tensor.transpose(gatT_ps[:], gat[:], i128f_s[:])
                gTb = wpool.tile([P, P], F32, tag="gTb")
                nc.scalar.activation(gTb[:], gatT_ps[:], AF.Identity, bias=bgc_s[:])
                y1_ps = psE.tile([P, P], F32, tag="epi")
                nc.tensor.matmul(out=y1_ps[:], lhsT=w1_s[:], rhs=gTb[:],
                                 start=True, stop=True)
                y1s = wpool.tile([P, P], F32, tag="y1s")
                nc.scalar.activation(y1s[:], y1_ps[:], AF.Relu, bias=b1c_s[:])
                y2_ps = psE.tile([P, P], F32, tag="epi")
                nc.tensor.matmul(out=y2_ps[:], lhsT=w2_s[:], rhs=y1s[:],
                                 start=True, stop=True)
                xo = wpool.tile([P, P], F32, tag="xo")
                nc.scalar.activation(xo[:], y2_ps[:], AF.Identity, bias=b2c_s[:])
                nc.sync.dma_start(out=xoutT[:, w * P : (w + 1) * P], in_=xo[:])

    nc.compile()
    return nc


# ----------------------------------------------------------------------------
# Host-side preprocessing
# ----------------------------------------------------------------------------

def _preprocess(edge_index, edge_attr, ncores, nloc, nwin):
    """Route/sort/pad edges per core into slot arrays.

    Slot s of window w holds one edge (or a pad): tile j = s // 128,
    edge lane q = s % 128. Returns per-core dicts with src_slot (for the
    per-layer host halo gather), one-hot Sel/SelT, and eaT, plus kt.
    """
    src = np.ascontiguousarray(edge_index[0]).astype(np.int64)
    dst = np.ascontiguousarray(edge_index[1]).astype(np.int64)
    n = nloc * ncores
    ea = np.ascontiguousarray(edge_attr, dtype=np.float32)

    deg = np.bincount(dst, minlength=n).astype(np.float32)
    order = np.argsort(dst, kind="stable")
    dst_s = dst[order]
    src_s = src[order]
    ea_s = ea[order]
    cs = np.concatenate(
        [np.zeros((1, ea.shape[1]), np.float64), np.cumsum(ea_s, 0, dtype=np.float64)]
    )
    starts = np.searchsorted(dst_s, np.arange(n))
    ends = np.searchsorted(dst_s, np.arange(n) + 1)
    loop_attr = ((cs[ends] - cs[starts]) / np.maximum(deg, 1.0)[:, None]).astype(
        np.float32
    )

    cores = []
    maxcnt = 0
    for c in range(ncores):
        base = c * nloc
        lo, hi = starts[base], ends[base + nloc - 1]
        s2 = np.concatenate([src_s[lo:hi], np.arange(base, base + nloc)])
        d2 = np.concatenate([dst_s[lo:hi], np.arange(base, base + nloc)]) - base
        e2 = np.concatenate([ea_s[lo:hi], loop_attr[base : base + nloc]], 0)
        o = np.argsort(d2, kind="stable")
        s2, d2, e2 = s2[o], d2[o], e2[o]
        win = d2 // P
        wstart = np.searchsorted(win, np.arange(nwin))
        wend = np.searchsorted(win, np.arange(nwin) + 1)
        cnts = wend - wstart
        maxcnt = max(maxcnt, int(cnts.max()))
        cores.append((s2, d2, e2, wstart, cnts))

    kt = -(-maxcnt // P)
    if kt % 2:
        kt += 1
    S = kt * P

    data = []
    for (s2, d2, e2, wstart, cnts) in cores:
        nslot = nwin * S
        src_slot = np.zeros(nslot, np.int64)
        dstw_slot = np.full(nslot, -1, np.int64)
        ea_slot = np.zeros((nslot, CDIM), np.float32)
        idx = np.concatenate([np.arange(cnts[w]) + w * S for w in range(nwin)])
        src_slot[idx] = s2
        dstw_slot[idx] = d2 % P
        ea_slot[idx] = e2

        dw = dstw_slot.reshape(nwin, kt, P)  # [w, j, q]
        sel = (dw[:, :, :, None] == np.arange(P)[None, None, None, :])
        sel = sel.transpose(0, 2, 1, 3).reshape(nwin, P, kt * P).astype(NP_SEL)
        selT = (dw[:, :, None, :] == np.arange(P)[None, None, :, None])
        selT = selT.transpose(0, 2, 1, 3).reshape(nwin, P, kt * P).astype(NP_SEL)
        eaT = np.ascontiguousarray(ea_slot.T).astype(NP_EDGE)
        data.append(dict(src_slot=src_slot, seld=sel, selTd=selT, eaT=eaT))
    return data, kt


def _layer_weight_maps(inputs, layer, att):
    """Shared (same for all cores) weight/const arrays for one layer."""
    i = layer
    attf = att[i].reshape(-1).astype(np.float32)  # [128]
    m = dict(
        Wl_b=np.ascontiguousarray(inputs["Wl"][i]).astype(NP_EDGE),
        Wr=np.ascontiguousarray(inputs["Wr"][i]).astype(NPF32),
        w1=np.ascontiguousarray(inputs["w1"][i]).astype(NPF32),
        w2=np.ascontiguousarray(inputs["w2"][i]).astype(NPF32),
        We_b=np.ascontiguousarray(inputs["We"][i]).astype(NP_EDGE),
        blv=np.asarray(inputs["bl"][i]).reshape(1, P).astype(NPF32),
        # bl + br both ride the per-dst xr one-hot expansion into z
        brv=(np.asarray(inputs["br"][i]) + np.asarray(inputs["bl"][i]))
        .reshape(1, P)
        .astype(NPF32),
        ones1=np.ones((1, P), NPF32),
        attb4=np.tile(attf[None, :], (P, 4)).astype(NP_EDGE),
        i08b=(0.8 * np.eye(P)).astype(NP_EDGE),
        i128f=np.eye(P, dtype=NPF32),
        b1c=np.asarray(inputs["b1"][i]).reshape(P, 1).astype(NPF32),
        b2c=np.asarray(inputs["b2"][i]).reshape(P, 1).astype(NPF32),
        bgc=np.asarray(inputs["bias"][i]).reshape(P, 1).astype(NPF32),
    )
    return m


_NC_CACHE = {}


def kernel(**inputs):
    nodes = np.asarray(inputs["nodes"], dtype=np.float32)
    edge_index = np.asarray(inputs["edge_index"])
    edge_attr = np.asarray(inputs["edge_attr"], dtype=np.float32)

    n, d = nodes.shape
    assert (n, d) == (N, D)
    nloc = n // NCORES
    nwin = -(-nloc // P)
    nlocp = nwin * P

    data, kt = _preprocess(edge_index, edge_attr, NCORES, nloc, nwin)

    key = (nwin, kt, NCORES)
    if key not in _NC_CACHE:
        _NC_CACHE[key] = build_layer_nc(dict(nwin=nwin, kt=kt, ncores=NCORES))
    nc = _NC_CACHE[key]

    x_curr = np.ascontiguousarray(nodes.T)  # [128, n] f32

    for layer in range(L):
        wmap = _layer_weight_maps(inputs, layer, np.asarray(inputs["att"]))
        xce = x_curr.astype(NP_EDGE)
        in_maps = []
        for c in range(NCORES):
            base = c * nloc
            xTloc = np.zeros((P, nlocp), NPF32)
            xTloc[:, :nloc] = x_curr[:, base : base + nloc]
            m = dict(wmap)
            m["xgT"] = np.ascontiguousarray(xce[:, data[c]["src_slot"]])
            m["xTloc"] = xTloc
            m["seld"] = data[c]["seld"]
            m["selTd"] = data[c]["selTd"]
            m["eaT"] = data[c]["eaT"]
            in_maps.append(m)
        res = run_bass_kernel_spmd(
            nc, in_maps, core_ids=list(range(NCORES)), trace=TRACE
        )
        if res.exec_time_ns is not None:
            LAST_EXEC_NS.append(res.exec_time_ns)
        if TRACE:
            LAST_RESULTS.append(res)
        outs = res.results
        x_next = np.zeros((P, n), NPF32)
        for c in range(NCORES):
            xo = outs[c]["xoutT"]
            x_next[:, c * nloc : (c + 1) * nloc] = xo[:, :nloc]
        x_curr = x_next

    return np.ascontiguousarray(x_curr.T.astype(np.float32))


# revision 19
# speedup vs baseline: 1.8931x; 1.8931x over previous
"""Bass/Trainium2 kernel for nn_BlockGNN (2-layer GATv2 + MLP) on 8 NeuronCores.

Strategy (per spec sharding hint):
  - Nodes partitioned across 8 cores by destination (6250 nodes/core).
  - Edges routed to the core owning their dst; self-loops appended; packed
    into 49 windows of 128 dst-nodes, each window padded to a uniform KT
    tiles of 128 edge slots (SPMD: same program on all cores).
  - Host performs the halo gather: for every edge slot it gathers the
    source node's raw features into a feature-major slab (x[src].T), which
    is uploaded per layer. One-hot Sel/SelT matrices (fp8) encode each
    slot's destination within its window.
  - Per layer (one SPMD launch per layer; host re-shards between layers),
    per 128-edge tile, on device:
      PE builds z = ea@We + (xr[dst] + bl + br) + xg@Wl in PSUM
        (SelT one-hot matmul expands xr; the slab is the matmul lhsT so
        the Wl projection happens on the tensor engine),
      leaky_relu folds in as z + 0.8*relu(-z) (ACT relu + scaled-identity
        matmul), attention logits via DVE mul+reduce, exp on ACT,
      a second PE pass computes the value projection xl = xg@Wl + bl,
      and one segment-reduction matmul per tile accumulates
        [sum ex*xl | sum ex] per dst node into PSUM.
  - Node stage per window: divide by denominators, +bias, PE transpose,
    2-matmul MLP, write the local slice of the next layer's features.
"""

import os
import sys
import time

import numpy as np

os.environ.setdefault("MYCRO_LOCAL_CACHE", "1")

for _p in ("/opt/trn_rl_repo",):
    if os.path.isdir(_p) and _p not in sys.path:
        sys.path.append(_p)

import concourse.bass as bass
import concourse.bacc as bacc
import concourse.mybir as mybir
import concourse.tile as tile
from concourse.bass import AP
from concourse.bass_utils import run_bass_kernel_spmd

F32 = mybir.dt.float32
BF16 = mybir.dt.bfloat16
FP8 = mybir.dt.float8e4
I32 = mybir.dt.int32

NPF32 = np.float32
NPBF16 = mybir.dt.np(BF16)
NPFP8 = mybir.dt.np(FP8)

# Problem constants
N, E, D, H, CDIM, L = 50000, 800000, 128, 4, 32, 2
P = 128
NCORES = 8
NEG = 0.2

# dtype knobs
_EDT = os.environ.get("KDT_EDGE", "bf16")
_SDT = os.environ.get("KDT_SEL", "fp8")
EDGE_DT = {"bf16": BF16, "f32": F32}[_EDT]
NP_EDGE = mybir.dt.np(EDGE_DT)
SEL_DT = {"fp8": FP8, "bf16": BF16, "f32": F32}[_SDT]
NP_SEL = mybir.dt.np(SEL_DT)
TRACE = bool(int(os.environ.get("KTRACE", "0")))

LAST_EXEC_NS = []     # per-launch exec_time_ns when tracing
LAST_RESULTS = []     # per-launch BassKernelResults when tracing


def _install_ntff_hook():
    """Provide antenv.axon_hooks (NTFF profiling via the axon PJRT .so)
    when the image lacks it, so run_bass_kernel_spmd(trace=True) works."""
    try:
        import antenv.axon_hooks  # noqa: F401
        return
    except ImportError:
        pass
    import contextlib
    import ctypes
    import types

    try:
        import antenv
    except ImportError:
        return
    so_path = "/opt/axon/libaxon_pjrt.so"
    if not os.path.exists(so_path):
        return
    lib = ctypes.CDLL(so_path)
    if not hasattr(lib, "axon_start_nrt_profile"):
        return
    lib.axon_start_nrt_profile.argtypes = [
        ctypes.POINTER(ctypes.c_int64),
        ctypes.c_size_t,
    ]
    lib.axon_start_nrt_profile.restype = ctypes.c_int64
    lib.axon_stop_nrt_profile.argtypes = [ctypes.c_char_p]
    lib.axon_stop_nrt_profile.restype = ctypes.c_int64

    @contextlib.contextmanager
    def _hook(output_dir, device_ids):
        import jax

        jax.devices()
        if device_ids:
            ids = (ctypes.c_int64 * len(device_ids))(*device_ids)
            rc = lib.axon_start_nrt_profile(ids, len(device_ids))
        else:
            rc = lib.axon_start_nrt_profile(None, 0)
        if rc != 0:
            raise RuntimeError(f"axon_start_nrt_profile rc={rc}")
        try:
            yield
        finally:
            n = lib.axon_stop_nrt_profile(str(output_dir).encode())
            print(f"ntff profile: {n} file(s) -> {output_dir}", file=sys.stderr)

    mod = types.ModuleType("antenv.axon_hooks")
    _state = {"hook": _hook}
    mod.get_axon_ntff_profile_hook = lambda: _state["hook"]
    mod.set_axon_ntff_profile_hook = lambda h: _state.update(hook=h)
    sys.modules["antenv.axon_hooks"] = mod
    antenv.axon_hooks = mod


if TRACE:
    _install_ntff_hook()


def _bcast_last(ap: AP, n: int) -> AP:
    """Append a stride-0 trailing dim of size n to an AP."""
    return AP(ap.tensor, ap.offset, [list(p) for p in ap.ap] + [[0, n]])


def build_layer_nc(cfg, enable_asserts=False):
    """Build the single-layer SPMD program. cfg: nwin, kt (tiles/window)."""
    NWIN, KT = cfg["nwin"], cfg["kt"]
    NLOCP = NWIN * P
    ESLOT = NWIN * KT * P
    assert KT % 2 == 0
    MACROS = []
    j0 = 0
    while j0 < KT:
        wdt = 4 if KT - j0 >= 4 else KT - j0
        MACROS.append((j0, wdt))
        j0 += wdt

    nc = bacc.Bacc(
        "TRN2",
        target_bir_lowering=False,
        debug=False,
        enable_asserts=enable_asserts,
        num_devices=cfg.get("ncores", NCORES),
    )

    # ---- inputs ----
    xgT = nc.dram_tensor("xgT", [P, ESLOT], EDGE_DT, kind="ExternalInput").ap()
    xTloc = nc.dram_tensor("xTloc", [P, NLOCP], F32, kind="ExternalInput").ap()
    Wl_b = nc.dram_tensor("Wl_b", [P, P], EDGE_DT, kind="ExternalInput").ap()
    Wr = nc.dram_tensor("Wr", [P, P], F32, kind="ExternalInput").ap()
    w1 = nc.dram_tensor("w1", [P, P], F32, kind="ExternalInput").ap()
    w2 = nc.dram_tensor("w2", [P, P], F32, kind="ExternalInput").ap()
    We_b = nc.dram_tensor("We_b", [CDIM, P], EDGE_DT, kind="ExternalInput").ap()
    brv = nc.dram_tensor("brv", [1, P], F32, kind="ExternalInput").ap()
    ones1 = nc.dram_tensor("ones1", [1, P], F32, kind="ExternalInput").ap()
    attb4 = nc.dram_tensor("attb4", [P, 4 * P], EDGE_DT, kind="ExternalInput").ap()
    i128f = nc.dram_tensor("i128f", [P, P], F32, kind="ExternalInput").ap()
    b1c = nc.dram_tensor("b1c", [P, 1], F32, kind="ExternalInput").ap()
    b2c = nc.dram_tensor("b2c", [P, 1], F32, kind="ExternalInput").ap()
    bgc = nc.dram_tensor("bgc", [P, 1], F32, kind="ExternalInput").ap()
    eaT = nc.dram_tensor("eaT", [CDIM, ESLOT], EDGE_DT, kind="ExternalInput").ap()
    seld = nc.dram_tensor("seld", [NWIN, P, KT * P], SEL_DT, kind="ExternalInput").ap()
    selTd = nc.dram_tensor("selTd", [NWIN, P, KT * P], SEL_DT, kind="ExternalInput").ap()
    xoutT = nc.dram_tensor("xoutT", [P, NLOCP], F32, kind="ExternalOutput").ap()

    AF = mybir.ActivationFunctionType
    OP = mybir.AluOpType

    with tile.TileContext(nc) as tc:
        with (
            tc.tile_pool(name="const", bufs=1) as cpool,
            tc.tile_pool(name="win", bufs=2) as wpool,
            tc.tile_pool(name="edge", bufs=3) as epool,
            tc.tile_pool(name="psZ", bufs=2, space="PSUM") as psZ,
            tc.tile_pool(name="psV", bufs=2, space="PSUM") as psV,
            tc.tile_pool(name="psO", bufs=1, space="PSUM") as psO,
            tc.tile_pool(name="psE", bufs=1, space="PSUM") as psE,
        ):
            # ---- load constants/weights to SBUF ----
            def cload(ap, shape, dt, tag):
                t = cpool.tile(shape, dt, tag=tag)
                nc.sync.dma_start(out=t[:], in_=ap)
                return t

            Wlb_s = cload(Wl_b, [P, P], EDGE_DT, tag="Wlb_s")
            Wr_s = cload(Wr, [P, P], F32, tag="Wr_s")
            w1_s = cload(w1, [P, P], F32, tag="w1_s")
            w2_s = cload(w2, [P, P], F32, tag="w2_s")
            We_s = cload(We_b, [CDIM, P], EDGE_DT, tag="We_s")
            brv_s = cload(brv, [1, P], F32, tag="brv_s")
            ones_s = cload(ones1, [1, P], F32, tag="ones_s")
            attb_s = cload(attb4, [P, 4 * P], EDGE_DT, tag="attb_s")
            i128f_s = cload(i128f, [P, P], F32, tag="i128f_s")
            b1c_s = cload(b1c, [P, 1], F32, tag="b1c_s")
            b2c_s = cload(b2c, [P, 1], F32, tag="b2c_s")
            bgc_s = cload(bgc, [P, 1], F32, tag="bgc_s")

            # ---- edge + node stage, per window ----
            for w in range(NWIN):
                xgT_sb = wpool.tile([P, KT * P], EDGE_DT, tag="xgT")
                nc.sync.dma_start(
                    out=xgT_sb[:], in_=xgT[:, w * KT * P : (w + 1) * KT * P]
                )
                eaT_sb = wpool.tile([CDIM, KT * P], EDGE_DT, tag="ea")
                nc.sync.dma_start(
                    out=eaT_sb[:], in_=eaT[:, w * KT * P : (w + 1) * KT * P]
                )
                sel_sb = wpool.tile([P, KT * P], SEL_DT, tag="sel")
                nc.sync.dma_start(out=sel_sb[:], in_=seld[w])
                selT_sb = wpool.tile([P, KT * P], SEL_DT, tag="selT")
                nc.sync.dma_start(out=selT_sb[:], in_=selTd[w])

                # xr for this window's 128 local nodes (+ bl + br folded in)
                xtl_sb = wpool.tile([P, P], F32, tag="xtl")
                nc.sync.dma_start(out=xtl_sb[:], in_=xTloc[:, w * P : (w + 1) * P])
                xr_ps = psE.tile([P, P], F32, tag="xr")
                nc.tensor.matmul(out=xr_ps[:], lhsT=xtl_sb[:], rhs=Wr_s[:],
                                 start=True, stop=False)
                nc.tensor.matmul(out=xr_ps[:], lhsT=ones_s[:], rhs=brv_s[:],
                                 start=False, stop=True)
                xr_sb = wpool.tile([P, P], EDGE_DT, tag="xrs")
                nc.scalar.activation(xr_sb[:], xr_ps[:], AF.Copy)

                out12 = psO.tile([P, 132], F32, tag="o12")

                for mi, (j0, MW) in enumerate(MACROS):
                    zq = psZ.tile([P, MW * P], F32, tag="zq")
                    vq = psV.tile([P, MW * P], F32, tag="vq")
                    # z = ea@We + xr[dst] + xg@Wl  accumulated in psum
                    for u in range(MW):
                        j = j0 + u
                        nc.tensor.matmul(
                            out=zq[:, u * P : (u + 1) * P],
                            lhsT=eaT_sb[:, j * P : (j + 1) * P],
                            rhs=We_s[:],
                            start=(u == 0),
                            stop=False,
                        )
                    for u in range(MW):
                        j = j0 + u
                        nc.tensor.matmul(
                            out=zq[:, u * P : (u + 1) * P],
                            lhsT=selT_sb[:, j * P : (j + 1) * P],
                            rhs=xr_sb[:],
                            start=False,
                            stop=False,
                        )
                    for u in range(MW):
                        j = j0 + u
                        nc.tensor.matmul(
                            out=zq[:, u * P : (u + 1) * P],
                            lhsT=xgT_sb[:, j * P : (j + 1) * P],
                            rhs=Wlb_s[:],
                            start=False,
                            stop=(u == MW - 1),
                        )
                    # value projection xl = xg@Wl (edge-major psum; bl's
                    # softmax-weighted contribution is bl*denom/(denom+eps)
                    # ~= bl, added with the gat bias in the epilogue)
                    for u in range(MW):
                        j = j0 + u
                        nc.tensor.matmul(
                            out=vq[:, u * P : (u + 1) * P],
                            lhsT=xgT_sb[:, j * P : (j + 1) * P],
                            rhs=Wlb_s[:],
                            start=(u == 0),
                            stop=(u == MW - 1),
                        )
                    # leaky = max(z, 0.2z), then alpha = per-head dot(att, m)
                    t02 = epool.tile([P, MW * P], EDGE_DT, tag="t02")
                    nc.scalar.activation(t02[:], zq[:], AF.Copy, scale=NEG)
                    m_sb = epool.tile([P, MW * P], EDGE_DT, tag="m_sb")
                    nc.vector.tensor_tensor(m_sb[:], zq[:], t02[:], op=OP.max)
                    am = epool.tile([P, MW * P], EDGE_DT, tag="am")
                    nc.vector.tensor_tensor(am[:], m_sb[:], attb_s[:, : MW * P],
                                            op=OP.mult)
                    alpha = epool.tile([P, 4 * MW], F32, tag="alpha")
                    nc.vector.tensor_reduce(
                        alpha[:],
                        am[:].rearrange("p (g c) -> p g c", c=CDIM),
                        mybir.AxisListType.X,
                        OP.add,
                    )
                    comb = epool.tile([P, MW * 132], EDGE_DT, tag="comb")
                    comb_v = comb[:].rearrange("p (b f) -> p b f", f=132)
                    nc.scalar.activation(
                        comb_v[:, :, P : P + 4],
                        alpha[:].rearrange("p (b h) -> p b h", h=4),
                        AF.Exp,
                    )
                    nc.vector.tensor_tensor(
                        comb_v[:, :, 0:P].rearrange("p b (h c) -> p b h c", c=CDIM),
                        vq[:].rearrange("p (b h c) -> p b h c", b=MW, c=CDIM),
                        _bcast_last(comb_v[:, :, P : P + 4], CDIM),
                        op=OP.mult,
                    )
                    for u in range(MW):
                        j = j0 + u
                        nc.tensor.matmul(
                            out=out12[:],
                            lhsT=sel_sb[:, j * P : (j + 1) * P],
                            rhs=comb[:, u * 132 : (u + 1) * 132],
                            start=(mi == 0 and u == 0),
                            stop=(mi == len(MACROS) - 1 and u == MW - 1),
                        )

                # ---- window epilogue ----
                de = wpool.tile([P, 4], F32, tag="de")
                nc.vector.tensor_scalar(de[:], out12[:, P : P + 4], 1e-16, None,
                                        OP.add)
                rc = wpool.tile([P, 4], F32, tag="rc")
                nc.vector.reciprocal(rc[:], de[:])
                gat = wpool.tile([P, P], F32, tag="gat")
                for h in range(H):
                    nc.vector.tensor_scalar(
                        gat[:, h * CDIM : (h + 1) * CDIM],
                        out12[:, h * CDIM : (h + 1) * CDIM],
                        rc[:, h : h + 1],
                        None,
                        OP.mult,
                    )
                gatT_ps = psE.tile([P, P], F32, tag="epi")
                nc.tensor.transpose(gatT_ps[:], gat[:], i128f_s[:])
                gTb = wpool.tile([P, P], F32, tag="gTb")
                nc.scalar.activation(gTb[:], gatT_ps[:], AF.Identity, bias=bgc_s[:])
                y1_ps = psE.tile([P, P], F32, tag="epi")
                nc.tensor.matmul(out=y1_ps[:], lhsT=w1_s[:], rhs=gTb[:],
                                 start=True, stop=True)
                y1s = wpool.tile([P, P], F32, tag="y1s")
                nc.scalar.activation(y1s[:], y1_ps[:], AF.Relu, bias=b1c_s[:])
                y2_ps = psE.tile([P, P], F32, tag="epi")
                nc.tensor.matmul(out=y2_ps[:], lhsT=w2_s[:], rhs=y1s[:],
                                 start=True, stop=True)
                xo = wpool.tile([P, P], F32, tag="xo")
                nc.scalar.activation(xo[:], y2_ps[:], AF.Identity, bias=b2c_s[:])
                nc.sync.dma_start(out=xoutT[:, w * P : (w + 1) * P], in_=xo[:])

    nc.compile()
    return nc


# ----------------------------------------------------------------------------
# Host-side preprocessing
# ----------------------------------------------------------------------------

def _preprocess(edge_index, edge_attr, ncores, nloc, nwin):
    """Route/sort/pad edges per core into slot arrays.

    Slot s of window w holds one edge (or a pad): tile j = s // 128,
    edge lane q = s % 128. Returns per-core dicts with src_slot (for the
    per-layer host halo gather), one-hot Sel/SelT, and eaT, plus kt.
    """
    src = np.ascontiguousarray(edge_index[0]).astype(np.int64)
    dst = np.ascontiguousarray(edge_index[1]).astype(np.int64)
    n = nloc * ncores
    ea = np.ascontiguousarray(edge_attr, dtype=np.float32)

    deg = np.bincount(dst, minlength=n).astype(np.float32)
    order = np.argsort(dst, kind="stable")
    dst_s = dst[order]
    src_s = src[order]
    ea_s = ea[order]
    cs = np.concatenate(
        [np.zeros((1, ea.shape[1]), np.float64), np.cumsum(ea_s, 0, dtype=np.float64)]
    )
    starts = np.searchsorted(dst_s, np.arange(n))
    ends = np.searchsorted(dst_s, np.arange(n) + 1)
    loop_attr = ((cs[ends] - cs[starts]) / np.maximum(deg, 1.0)[:, None]).astype(
        np.float32
    )

    cores = []
    maxcnt = 0
    for c in range(ncores):
        base = c * nloc
        lo, hi = starts[base], ends[base + nloc - 1]
        s2 = np.concatenate([src_s[lo:hi], np.arange(base, base + nloc)])
        d2 = np.concatenate([dst_s[lo:hi], np.arange(base, base + nloc)]) - base
        e2 = np.concatenate([ea_s[lo:hi], loop_attr[base : base + nloc]], 0)
        o = np.argsort(d2, kind="stable")
        s2, d2, e2 = s2[o], d2[o], e2[o]
        win = d2 // P
        wstart = np.searchsorted(win, np.arange(nwin))
        wend = np.searchsorted(win, np.arange(nwin) + 1)
        cnts = wend - wstart
        maxcnt = max(maxcnt, int(cnts.max()))
        cores.append((s2, d2, e2, wstart, cnts))

    kt = -(-maxcnt // P)
    if kt % 2:
        kt += 1
    S = kt * P

    data = []
    for (s2, d2, e2, wstart, cnts) in cores:
        nslot = nwin * S
        src_slot = np.zeros(nslot, np.int64)
        dstw_slot = np.full(nslot, -1, np.int64)
        ea_slot = np.zeros((nslot, CDIM), np.float32)
        idx = np.concatenate([np.arange(cnts[w]) + w * S for w in range(nwin)])
        src_slot[idx] = s2
        dstw_slot[idx] = d2 % P
        ea_slot[idx] = e2

        dw = dstw_slot.reshape(nwin, kt, P)  # [w, j, q]
        sel = (dw[:, :, :, None] == np.arange(P)[None, None, None, :])
        sel = sel.transpose(0, 2, 1, 3).reshape(nwin, P, kt * P).astype(NP_SEL)
        selT = (dw[:, :, None, :] == np.arange(P)[None, None, :, None])
        selT = selT.transpose(0, 2, 1, 3).reshape(nwin, P, kt * P).astype(NP_SEL)
        eaT = np.ascontiguousarray(ea_slot.T).astype(NP_EDGE)
        data.append(dict(src_slot=src_slot, seld=sel, selTd=selT, eaT=eaT))
    return data, kt


def _layer_weight_maps(inputs, layer, att):
    """Shared (same for all cores) weight/const arrays for one layer."""
    i = layer
    attf = att[i].reshape(-1).astype(np.float32)  # [128]
    m = dict(
        Wl_b=np.ascontiguousarray(inputs["Wl"][i]).astype(NP_EDGE),
        Wr=np.ascontiguousarray(inputs["Wr"][i]).astype(NPF32),
        w1=np.ascontiguousarray(inputs["w1"][i]).astype(NPF32),
        w2=np.ascontiguousarray(inputs["w2"][i]).astype(NPF32),
        We_b=np.ascontiguousarray(inputs["We"][i]).astype(NP_EDGE),
        # bl + br both ride the per-dst xr one-hot expansion into z
        brv=(np.asarray(inputs["br"][i]) + np.asarray(inputs["bl"][i]))
        .reshape(1, P)
        .astype(NPF32),
        ones1=np.ones((1, P), NPF32),
        attb4=np.tile(attf[None, :], (P, 4)).astype(NP_EDGE),
        i128f=np.eye(P, dtype=NPF32),
        b1c=np.asarray(inputs["b1"][i]).reshape(P, 1).astype(NPF32),
        b2c=np.asarray(inputs["b2"][i]).reshape(P, 1).astype(NPF32),
        # gat bias + bl (bl rides the normalized softmax weights, sum ~= 1)
        bgc=(np.asarray(inputs["bias"][i]) + np.asarray(inputs["bl"][i]))
        .reshape(P, 1)
        .astype(NPF32),
    )
    return m


_NC_CACHE = {}


def kernel(**inputs):
    nodes = np.asarray(inputs["nodes"], dtype=np.float32)
    edge_index = np.asarray(inputs["edge_index"])
    edge_attr = np.asarray(inputs["edge_attr"], dtype=np.float32)

    n, d = nodes.shape
    assert (n, d) == (N, D)
    nloc = n // NCORES
    nwin = -(-nloc // P)
    nlocp = nwin * P

    data, kt = _preprocess(edge_index, edge_attr, NCORES, nloc, nwin)

    key = (nwin, kt, NCORES)
    if key not in _NC_CACHE:
        _NC_CACHE[key] = build_layer_nc(dict(nwin=nwin, kt=kt, ncores=NCORES))
    nc = _NC_CACHE[key]

    x_curr = np.ascontiguousarray(nodes.T)  # [128, n] f32

    for layer in range(L):
        wmap = _layer_weight_maps(inputs, layer, np.asarray(inputs["att"]))
        xce = x_curr.astype(NP_EDGE)
        in_maps = []
        for c in range(NCORES):
            base = c * nloc
            xTloc = np.zeros((P, nlocp), NPF32)
            xTloc[:, :nloc] = x_curr[:, base : base + nloc]
            m = dict(wmap)
            m["xgT"] = np.ascontiguousarray(xce[:, data[c]["src_slot"]])
            m["xTloc"] = xTloc
            m["seld"] = data[c]["seld"]
            m["selTd"] = data[c]["selTd"]
            m["eaT"] = data[c]["eaT"]
            in_maps.append(m)
        res = run_bass_kernel_spmd(
            nc, in_maps, core_ids=list(range(NCORES)), trace=TRACE
        )
        if res.exec_time_ns is not None:
            LAST_EXEC_NS.append(res.exec_time_ns)
        if TRACE:
            LAST_RESULTS.append(res)
        outs = res.results
        x_next = np.zeros((P, n), NPF32)
        for c in range(NCORES):
            xo = outs[c]["xoutT"]
            x_next[:, c * nloc : (c + 1) * nloc] = xo[:, :nloc]
        x_curr = x_next

    return np.ascontiguousarray(x_curr.T.astype(np.float32))


# revision 20
# speedup vs baseline: 2.1845x; 1.1540x over previous
"""Bass/Trainium2 kernel for nn_BlockGNN (2-layer GATv2 + MLP) on 8 NeuronCores.

Strategy (per spec sharding hint):
  - Nodes partitioned across 8 cores by destination (6250 nodes/core).
  - Edges routed to the core owning their dst; self-loops appended; packed
    into 49 windows of 128 dst-nodes, each window padded to a uniform KT
    tiles of 128 edge slots (SPMD: same program on all cores).
  - Host performs the halo gather: for every edge slot it gathers the
    source node's raw features into a feature-major slab (x[src].T), which
    is uploaded per layer. One-hot Sel/SelT matrices (fp8) encode each
    slot's destination within its window.
  - Per layer (one SPMD launch per layer; host re-shards between layers),
    per 128-edge tile, on device:
      PE builds z = ea@We + (xr[dst] + bl + br) + xg@Wl in PSUM
        (SelT one-hot matmul expands xr; the slab is the matmul lhsT so
        the Wl projection happens on the tensor engine),
      leaky_relu folds in as z + 0.8*relu(-z) (ACT relu + scaled-identity
        matmul), attention logits via DVE mul+reduce, exp on ACT,
      a second PE pass computes the value projection xl = xg@Wl + bl,
      and one segment-reduction matmul per tile accumulates
        [sum ex*xl | sum ex] per dst node into PSUM.
  - Node stage per window: divide by denominators, +bias, PE transpose,
    2-matmul MLP, write the local slice of the next layer's features.
"""

import os
import sys
import time

import numpy as np

os.environ.setdefault("MYCRO_LOCAL_CACHE", "1")

for _p in ("/opt/trn_rl_repo",):
    if os.path.isdir(_p) and _p not in sys.path:
        sys.path.append(_p)

import concourse.bass as bass
import concourse.bacc as bacc
import concourse.mybir as mybir
import concourse.tile as tile
from concourse.bass import AP
from concourse.bass_utils import run_bass_kernel_spmd

F32 = mybir.dt.float32
BF16 = mybir.dt.bfloat16
FP8 = mybir.dt.float8e4
I32 = mybir.dt.int32

NPF32 = np.float32
NPBF16 = mybir.dt.np(BF16)
NPFP8 = mybir.dt.np(FP8)

# Problem constants
N, E, D, H, CDIM, L = 50000, 800000, 128, 4, 32, 2
P = 128
NCORES = 8
NEG = 0.2

# dtype knobs
_EDT = os.environ.get("KDT_EDGE", "bf16")
_SDT = os.environ.get("KDT_SEL", "fp8")
EDGE_DT = {"bf16": BF16, "f32": F32}[_EDT]
NP_EDGE = mybir.dt.np(EDGE_DT)
SEL_DT = {"fp8": FP8, "bf16": BF16, "f32": F32}[_SDT]
NP_SEL = mybir.dt.np(SEL_DT)
TRACE = bool(int(os.environ.get("KTRACE", "0")))

LAST_EXEC_NS = []     # per-launch exec_time_ns when tracing
LAST_RESULTS = []     # per-launch BassKernelResults when tracing


def _install_ntff_hook():
    """Provide antenv.axon_hooks (NTFF profiling via the axon PJRT .so)
    when the image lacks it, so run_bass_kernel_spmd(trace=True) works."""
    try:
        import antenv.axon_hooks  # noqa: F401
        return
    except ImportError:
        pass
    import contextlib
    import ctypes
    import types

    try:
        import antenv
    except ImportError:
        return
    so_path = "/opt/axon/libaxon_pjrt.so"
    if not os.path.exists(so_path):
        return
    lib = ctypes.CDLL(so_path)
    if not hasattr(lib, "axon_start_nrt_profile"):
        return
    lib.axon_start_nrt_profile.argtypes = [
        ctypes.POINTER(ctypes.c_int64),
        ctypes.c_size_t,
    ]
    lib.axon_start_nrt_profile.restype = ctypes.c_int64
    lib.axon_stop_nrt_profile.argtypes = [ctypes.c_char_p]
    lib.axon_stop_nrt_profile.restype = ctypes.c_int64

    @contextlib.contextmanager
    def _hook(output_dir, device_ids):
        import jax

        jax.devices()
        if device_ids:
            ids = (ctypes.c_int64 * len(device_ids))(*device_ids)
            rc = lib.axon_start_nrt_profile(ids, len(device_ids))
        else:
            rc = lib.axon_start_nrt_profile(None, 0)
        if rc != 0:
            raise RuntimeError(f"axon_start_nrt_profile rc={rc}")
        try:
            yield
        finally:
            n = lib.axon_stop_nrt_profile(str(output_dir).encode())
            print(f"ntff profile: {n} file(s) -> {output_dir}", file=sys.stderr)

    mod = types.ModuleType("antenv.axon_hooks")
    _state = {"hook": _hook}
    mod.get_axon_ntff_profile_hook = lambda: _state["hook"]
    mod.set_axon_ntff_profile_hook = lambda h: _state.update(hook=h)
    sys.modules["antenv.axon_hooks"] = mod
    antenv.axon_hooks = mod


if TRACE:
    _install_ntff_hook()


def _bcast_last(ap: AP, n: int) -> AP:
    """Append a stride-0 trailing dim of size n to an AP."""
    return AP(ap.tensor, ap.offset, [list(p) for p in ap.ap] + [[0, n]])


def build_layer_nc(cfg, enable_asserts=False):
    """Build the single-layer SPMD program. cfg: nwin, kt (tiles/window)."""
    NWIN, KT = cfg["nwin"], cfg["kt"]
    NLOCP = NWIN * P
    ESLOT = NWIN * KT * P
    assert KT % 2 == 0
    MACROS = []
    j0 = 0
    while j0 < KT:
        wdt = 4 if KT - j0 >= 4 else KT - j0
        MACROS.append((j0, wdt))
        j0 += wdt

    nc = bacc.Bacc(
        "TRN2",
        target_bir_lowering=False,
        debug=False,
        enable_asserts=enable_asserts,
        num_devices=cfg.get("ncores", NCORES),
    )

    # ---- inputs ----
    xgT = nc.dram_tensor("xgT", [P, ESLOT], EDGE_DT, kind="ExternalInput").ap()
    xTloc = nc.dram_tensor("xTloc", [P, NLOCP], F32, kind="ExternalInput").ap()
    Wl_b = nc.dram_tensor("Wl_b", [P, P], EDGE_DT, kind="ExternalInput").ap()
    Wr = nc.dram_tensor("Wr", [P, P], F32, kind="ExternalInput").ap()
    w1 = nc.dram_tensor("w1", [P, P], F32, kind="ExternalInput").ap()
    w2 = nc.dram_tensor("w2", [P, P], F32, kind="ExternalInput").ap()
    We_b = nc.dram_tensor("We_b", [CDIM, P], EDGE_DT, kind="ExternalInput").ap()
    brv = nc.dram_tensor("brv", [1, P], F32, kind="ExternalInput").ap()
    ones1 = nc.dram_tensor("ones1", [1, P], F32, kind="ExternalInput").ap()
    attb4 = nc.dram_tensor("attb4", [P, 4 * P], EDGE_DT, kind="ExternalInput").ap()
    i128f = nc.dram_tensor("i128f", [P, P], F32, kind="ExternalInput").ap()
    b1c = nc.dram_tensor("b1c", [P, 1], F32, kind="ExternalInput").ap()
    b2c = nc.dram_tensor("b2c", [P, 1], F32, kind="ExternalInput").ap()
    bgc = nc.dram_tensor("bgc", [P, 1], F32, kind="ExternalInput").ap()
    eaT = nc.dram_tensor("eaT", [CDIM, ESLOT], EDGE_DT, kind="ExternalInput").ap()
    seld = nc.dram_tensor("seld", [NWIN, P, KT * P], SEL_DT, kind="ExternalInput").ap()
    selTd = nc.dram_tensor("selTd", [NWIN, P, KT * P], SEL_DT, kind="ExternalInput").ap()
    xoutT = nc.dram_tensor("xoutT", [P, NLOCP], F32, kind="ExternalOutput").ap()

    AF = mybir.ActivationFunctionType
    OP = mybir.AluOpType

    with tile.TileContext(nc) as tc:
        with (
            tc.tile_pool(name="const", bufs=1) as cpool,
            tc.tile_pool(name="win", bufs=2) as wpool,
            tc.tile_pool(name="edge", bufs=3) as epool,
            tc.tile_pool(name="psZ", bufs=2, space="PSUM") as psZ,
            tc.tile_pool(name="psV", bufs=2, space="PSUM") as psV,
            tc.tile_pool(name="psO", bufs=1, space="PSUM") as psO,
            tc.tile_pool(name="psE", bufs=1, space="PSUM") as psE,
        ):
            # ---- load constants/weights to SBUF ----
            def cload(ap, shape, dt, tag):
                t = cpool.tile(shape, dt, tag=tag)
                nc.sync.dma_start(out=t[:], in_=ap)
                return t

            Wlb_s = cload(Wl_b, [P, P], EDGE_DT, tag="Wlb_s")
            Wr_s = cload(Wr, [P, P], F32, tag="Wr_s")
            w1_s = cload(w1, [P, P], F32, tag="w1_s")
            w2_s = cload(w2, [P, P], F32, tag="w2_s")
            We_s = cload(We_b, [CDIM, P], EDGE_DT, tag="We_s")
            brv_s = cload(brv, [1, P], F32, tag="brv_s")
            ones_s = cload(ones1, [1, P], F32, tag="ones_s")
            attb_s = cload(attb4, [P, 4 * P], EDGE_DT, tag="attb_s")
            i128f_s = cload(i128f, [P, P], F32, tag="i128f_s")
            b1c_s = cload(b1c, [P, 1], F32, tag="b1c_s")
            b2c_s = cload(b2c, [P, 1], F32, tag="b2c_s")
            bgc_s = cload(bgc, [P, 1], F32, tag="bgc_s")

            # ---- edge + node stage, per window ----
            for w in range(NWIN):
                xgT_sb = wpool.tile([P, KT * P], EDGE_DT, tag="xgT")
                nc.sync.dma_start(
                    out=xgT_sb[:], in_=xgT[:, w * KT * P : (w + 1) * KT * P]
                )
                eaT_sb = wpool.tile([CDIM, KT * P], EDGE_DT, tag="ea")
                nc.sync.dma_start(
                    out=eaT_sb[:], in_=eaT[:, w * KT * P : (w + 1) * KT * P]
                )
                sel_sb = wpool.tile([P, KT * P], SEL_DT, tag="sel")
                nc.sync.dma_start(out=sel_sb[:], in_=seld[w])
                selT_sb = wpool.tile([P, KT * P], SEL_DT, tag="selT")
                nc.sync.dma_start(out=selT_sb[:], in_=selTd[w])

                # xr for this window's 128 local nodes (+ bl + br folded in)
                xtl_sb = wpool.tile([P, P], F32, tag="xtl")
                nc.sync.dma_start(out=xtl_sb[:], in_=xTloc[:, w * P : (w + 1) * P])
                xr_ps = psE.tile([P, P], F32, tag="xr")
                nc.tensor.matmul(out=xr_ps[:], lhsT=xtl_sb[:], rhs=Wr_s[:],
                                 start=True, stop=False)
                nc.tensor.matmul(out=xr_ps[:], lhsT=ones_s[:], rhs=brv_s[:],
                                 start=False, stop=True)
                xr_sb = wpool.tile([P, P], EDGE_DT, tag="xrs")
                nc.scalar.activation(xr_sb[:], xr_ps[:], AF.Copy)

                out12 = psO.tile([P, 132], F32, tag="o12")

                for mi, (j0, MW) in enumerate(MACROS):
                    zq = psZ.tile([P, MW * P], F32, tag="zq")
                    vq = psV.tile([P, MW * P], F32, tag="vq")
                    # z = ea@We + xr[dst] + xg@Wl  accumulated in psum
                    for u in range(MW):
                        j = j0 + u
                        nc.tensor.matmul(
                            out=zq[:, u * P : (u + 1) * P],
                            lhsT=eaT_sb[:, j * P : (j + 1) * P],
                            rhs=We_s[:],
                            start=(u == 0),
                            stop=False,
                        )
                    for u in range(MW):
                        j = j0 + u
                        nc.tensor.matmul(
                            out=zq[:, u * P : (u + 1) * P],
                            lhsT=selT_sb[:, j * P : (j + 1) * P],
                            rhs=xr_sb[:],
                            start=False,
                            stop=False,
                        )
                    for u in range(MW):
                        j = j0 + u
                        nc.tensor.matmul(
                            out=zq[:, u * P : (u + 1) * P],
                            lhsT=xgT_sb[:, j * P : (j + 1) * P],
                            rhs=Wlb_s[:],
                            start=False,
                            stop=(u == MW - 1),
                        )
                    # value projection xl = xg@Wl (edge-major psum; bl's
                    # softmax-weighted contribution is bl*denom/(denom+eps)
                    # ~= bl, added with the gat bias in the epilogue)
                    for u in range(MW):
                        j = j0 + u
                        nc.tensor.matmul(
                            out=vq[:, u * P : (u + 1) * P],
                            lhsT=xgT_sb[:, j * P : (j + 1) * P],
                            rhs=Wlb_s[:],
                            start=(u == 0),
                            stop=(u == MW - 1),
                        )
                    # leaky = max(z, 0.2z), then alpha = per-head dot(att, m)
                    t02 = epool.tile([P, MW * P], EDGE_DT, tag="t02")
                    nc.scalar.activation(t02[:], zq[:], AF.Copy, scale=NEG)
                    m_sb = epool.tile([P, MW * P], EDGE_DT, tag="m_sb")
                    nc.vector.tensor_tensor(m_sb[:], zq[:], t02[:], op=OP.max)
                    am = epool.tile([P, MW * P], EDGE_DT, tag="am")
                    nc.vector.tensor_tensor(am[:], m_sb[:], attb_s[:, : MW * P],
                                            op=OP.mult)
                    alpha = epool.tile([P, 4 * MW], F32, tag="alpha")
                    nc.vector.tensor_reduce(
                        alpha[:],
                        am[:].rearrange("p (g c) -> p g c", c=CDIM),
                        mybir.AxisListType.X,
                        OP.add,
                    )
                    comb = epool.tile([P, MW * 132], EDGE_DT, tag="comb")
                    comb_v = comb[:].rearrange("p (b f) -> p b f", f=132)
                    nc.scalar.activation(
                        comb_v[:, :, P : P + 4],
                        alpha[:].rearrange("p (b h) -> p b h", h=4),
                        AF.Exp,
                    )
                    nc.vector.tensor_tensor(
                        comb_v[:, :, 0:P].rearrange("p b (h c) -> p b h c", c=CDIM),
                        vq[:].rearrange("p (b h c) -> p b h c", b=MW, c=CDIM),
                        _bcast_last(comb_v[:, :, P : P + 4], CDIM),
                        op=OP.mult,
                    )
                    for u in range(MW):
                        j = j0 + u
                        nc.tensor.matmul(
                            out=out12[:],
                            lhsT=sel_sb[:, j * P : (j + 1) * P],
                            rhs=comb[:, u * 132 : (u + 1) * 132],
                            start=(mi == 0 and u == 0),
                            stop=(mi == len(MACROS) - 1 and u == MW - 1),
                        )

                # ---- window epilogue ----
                de = wpool.tile([P, 4], F32, tag="de")
                nc.vector.tensor_scalar(de[:], out12[:, P : P + 4], 1e-16, None,
                                        OP.add)
                rc = wpool.tile([P, 4], F32, tag="rc")
                nc.vector.reciprocal(rc[:], de[:])
                gat = wpool.tile([P, P], F32, tag="gat")
                for h in range(H):
                    nc.vector.tensor_scalar(
                        gat[:, h * CDIM : (h + 1) * CDIM],
                        out12[:, h * CDIM : (h + 1) * CDIM],
                        rc[:, h : h + 1],
                        None,
                        OP.mult,
                    )
                gatT_ps = psE.tile([P, P], F32, tag="epi")
                nc.tensor.transpose(gatT_ps[:], gat[:], i128f_s[:])
                gTb = wpool.tile([P, P], F32, tag="gTb")
                nc.scalar.activation(gTb[:], gatT_ps[:], AF.Identity, bias=bgc_s[:])
                y1_ps = psE.tile([P, P], F32, tag="epi")
                nc.tensor.matmul(out=y1_ps[:], lhsT=w1_s[:], rhs=gTb[:],
                                 start=True, stop=True)
                y1s = wpool.tile([P, P], F32, tag="y1s")
                nc.scalar.activation(y1s[:], y1_ps[:], AF.Relu, bias=b1c_s[:])
                y2_ps = psE.tile([P, P], F32, tag="epi")
                nc.tensor.matmul(out=y2_ps[:], lhsT=w2_s[:], rhs=y1s[:],
                                 start=True, stop=True)
                xo = wpool.tile([P, P], F32, tag="xo")
                nc.scalar.activation(xo[:], y2_ps[:], AF.Identity, bias=b2c_s[:])
                nc.sync.dma_start(out=xoutT[:, w * P : (w + 1) * P], in_=xo[:])

    nc.compile()
    return nc


# ----------------------------------------------------------------------------
# Host-side preprocessing
# ----------------------------------------------------------------------------

def _preprocess(edge_index, edge_attr, ncores, nloc, nwin):
    """Route edges per core into gather-ready slot arrays.

    Node->window assignment is degree-balanced (LPT) so every window has a
    near-equal edge count, minimizing the uniform KT tile budget. Slot s of
    window w: tile j = s // 128, edge lane q = s % 128. Returns per-core
    dicts with src_slot (for the per-layer host halo gather), one-hot
    Sel/SelT, eaT, and the node permutation, plus kt.
    """
    src = np.ascontiguousarray(edge_index[0]).astype(np.int64)
    dst = np.ascontiguousarray(edge_index[1]).astype(np.int64)
    n = nloc * ncores
    ea = np.ascontiguousarray(edge_attr, dtype=np.float32)

    deg = np.bincount(dst, minlength=n).astype(np.float32)
    order = np.argsort(dst, kind="stable")
    dst_s = dst[order]
    src_s = src[order]
    ea_s = ea[order]
    cs = np.concatenate(
        [np.zeros((1, ea.shape[1]), np.float64), np.cumsum(ea_s, 0, dtype=np.float64)]
    )
    starts = np.searchsorted(dst_s, np.arange(n))
    ends = np.searchsorted(dst_s, np.arange(n) + 1)
    loop_attr = ((cs[ends] - cs[starts]) / np.maximum(deg, 1.0)[:, None]).astype(
        np.float32
    )

    import heapq

    cores = []
    maxcnt = 0
    for c in range(ncores):
        base = c * nloc
        # edges of this core (dst-local), self-loops appended
        lo, hi = starts[base], ends[base + nloc - 1]
        s2 = np.concatenate([src_s[lo:hi], np.arange(base, base + nloc)])
        dl = np.concatenate([dst_s[lo:hi], np.arange(base, base + nloc)]) - base
        e2 = np.concatenate([ea_s[lo:hi], loop_attr[base : base + nloc]], 0)

        # LPT balance: assign local nodes (weight = deg+1) to nwin windows
        w_of = np.empty(nloc, np.int64)
        pos_of = np.empty(nloc, np.int64)
        wdeg = (deg[base : base + nloc] + 1.0).astype(np.int64)
        heap = [(0, w, 0) for w in range(nwin)]  # (total, window, fill)
        heapq.heapify(heap)
        for node in np.argsort(-wdeg):
            tot, w, fill = heapq.heappop(heap)
            w_of[node] = w
            pos_of[node] = fill
            fill += 1
            tot += int(wdeg[node])
            if fill < P:
                heapq.heappush(heap, (tot, w, fill))
            else:
                heapq.heappush(heap, (1 << 60, w, fill))
        we = w_of[dl]
        pe_ = pos_of[dl]
        o = np.argsort(we, kind="stable")
        s2, e2, we, pe_ = s2[o], e2[o], we[o], pe_[o]
        wstart = np.searchsorted(we, np.arange(nwin))
        wend = np.searchsorted(we, np.arange(nwin) + 1)
        cnts = wend - wstart
        maxcnt = max(maxcnt, int(cnts.max()))
        # node permutation: slot w*128+pos holds local node id
        nl_flat = np.zeros(nwin * P, np.int64)
        nl_flat[w_of * P + pos_of] = np.arange(nloc)
        used = np.zeros(nwin * P, bool)
        used[w_of * P + pos_of] = True
        cores.append((s2, e2, pe_, wstart, cnts, nl_flat, used))

    kt = -(-maxcnt // P)
    if kt % 2:
        kt += 1
    S = kt * P

    data = []
    for (s2, e2, pe_, wstart, cnts, nl_flat, used) in cores:
        nslot = nwin * S
        src_slot = np.zeros(nslot, np.int64)
        dstw_slot = np.full(nslot, -1, np.int64)
        ea_slot = np.zeros((nslot, CDIM), np.float32)
        idx = np.concatenate([np.arange(cnts[w]) + w * S for w in range(nwin)])
        src_slot[idx] = s2
        dstw_slot[idx] = pe_
        ea_slot[idx] = e2

        dw = dstw_slot.reshape(nwin, kt, P)  # [w, j, q]
        sel = (dw[:, :, :, None] == np.arange(P)[None, None, None, :])
        sel = sel.transpose(0, 2, 1, 3).reshape(nwin, P, kt * P).astype(NP_SEL)
        selT = (dw[:, :, None, :] == np.arange(P)[None, None, :, None])
        selT = selT.transpose(0, 2, 1, 3).reshape(nwin, P, kt * P).astype(NP_SEL)
        eaT = np.ascontiguousarray(ea_slot.T).astype(NP_EDGE)
        data.append(dict(src_slot=src_slot, seld=sel, selTd=selT, eaT=eaT,
                         nl_flat=nl_flat, used=used))
    return data, kt


def _layer_weight_maps(inputs, layer, att):
    """Shared (same for all cores) weight/const arrays for one layer."""
    i = layer
    attf = att[i].reshape(-1).astype(np.float32)  # [128]
    m = dict(
        Wl_b=np.ascontiguousarray(inputs["Wl"][i]).astype(NP_EDGE),
        Wr=np.ascontiguousarray(inputs["Wr"][i]).astype(NPF32),
        w1=np.ascontiguousarray(inputs["w1"][i]).astype(NPF32),
        w2=np.ascontiguousarray(inputs["w2"][i]).astype(NPF32),
        We_b=np.ascontiguousarray(inputs["We"][i]).astype(NP_EDGE),
        # bl + br both ride the per-dst xr one-hot expansion into z
        brv=(np.asarray(inputs["br"][i]) + np.asarray(inputs["bl"][i]))
        .reshape(1, P)
        .astype(NPF32),
        ones1=np.ones((1, P), NPF32),
        attb4=np.tile(attf[None, :], (P, 4)).astype(NP_EDGE),
        i128f=np.eye(P, dtype=NPF32),
        b1c=np.asarray(inputs["b1"][i]).reshape(P, 1).astype(NPF32),
        b2c=np.asarray(inputs["b2"][i]).reshape(P, 1).astype(NPF32),
        # gat bias + bl (bl rides the normalized softmax weights, sum ~= 1)
        bgc=(np.asarray(inputs["bias"][i]) + np.asarray(inputs["bl"][i]))
        .reshape(P, 1)
        .astype(NPF32),
    )
    return m


_NC_CACHE = {}


def kernel(**inputs):
    nodes = np.asarray(inputs["nodes"], dtype=np.float32)
    edge_index = np.asarray(inputs["edge_index"])
    edge_attr = np.asarray(inputs["edge_attr"], dtype=np.float32)

    n, d = nodes.shape
    assert (n, d) == (N, D)
    nloc = n // NCORES
    nwin = -(-nloc // P)
    nlocp = nwin * P

    data, kt = _preprocess(edge_index, edge_attr, NCORES, nloc, nwin)

    key = (nwin, kt, NCORES)
    if key not in _NC_CACHE:
        _NC_CACHE[key] = build_layer_nc(dict(nwin=nwin, kt=kt, ncores=NCORES))
    nc = _NC_CACHE[key]

    x_curr = np.ascontiguousarray(nodes.T)  # [128, n] f32

    for layer in range(L):
        wmap = _layer_weight_maps(inputs, layer, np.asarray(inputs["att"]))
        xce = x_curr.astype(NP_EDGE)
        in_maps = []
        for c in range(NCORES):
            base = c * nloc
            xTloc = x_curr[:, base + data[c]["nl_flat"]].copy()
            xTloc[:, ~data[c]["used"]] = 0.0
            m = dict(wmap)
            m["xgT"] = np.ascontiguousarray(xce[:, data[c]["src_slot"]])
            m["xTloc"] = xTloc
            m["seld"] = data[c]["seld"]
            m["selTd"] = data[c]["selTd"]
            m["eaT"] = data[c]["eaT"]
            in_maps.append(m)
        res = run_bass_kernel_spmd(
            nc, in_maps, core_ids=list(range(NCORES)), trace=TRACE
        )
        if res.exec_time_ns is not None:
            LAST_EXEC_NS.append(res.exec_time_ns)
        if TRACE:
            LAST_RESULTS.append(res)
        outs = res.results
        x_next = np.zeros((P, n), NPF32)
        for c in range(NCORES):
            xo = outs[c]["xoutT"]
            u = data[c]["used"]
            x_next[:, c * nloc + data[c]["nl_flat"][u]] = xo[:, u]
        x_curr = x_next

    return np.ascontiguousarray(x_curr.T.astype(np.float32))


# revision 23
# speedup vs baseline: 2.1846x; 1.0000x over previous
"""Bass/Trainium2 kernel for nn_BlockGNN (2-layer GATv2 + MLP) on 8 NeuronCores.

Strategy (per spec sharding hint):
  - Nodes partitioned across 8 cores by destination (6250 nodes/core).
  - Edges routed to the core owning their dst; self-loops appended; packed
    into 49 windows of 128 dst-nodes, each window padded to a uniform KT
    tiles of 128 edge slots (SPMD: same program on all cores).
  - Host performs the halo gather: for every edge slot it gathers the
    source node's raw features into a feature-major slab (x[src].T), which
    is uploaded per layer. One-hot Sel/SelT matrices (fp8) encode each
    slot's destination within its window.
  - Per layer (one SPMD launch per layer; host re-shards between layers),
    per 128-edge tile, on device:
      PE builds z = ea@We + (xr[dst] + bl + br) + xg@Wl in PSUM
        (SelT one-hot matmul expands xr; the slab is the matmul lhsT so
        the Wl projection happens on the tensor engine),
      leaky_relu folds in as z + 0.8*relu(-z) (ACT relu + scaled-identity
        matmul), attention logits via DVE mul+reduce, exp on ACT,
      a second PE pass computes the value projection xl = xg@Wl + bl,
      and one segment-reduction matmul per tile accumulates
        [sum ex*xl | sum ex] per dst node into PSUM.
  - Node stage per window: divide by denominators, +bias, PE transpose,
    2-matmul MLP, write the local slice of the next layer's features.
"""

import os
import sys
import time

import numpy as np

os.environ.setdefault("MYCRO_LOCAL_CACHE", "1")

for _p in ("/opt/trn_rl_repo",):
    if os.path.isdir(_p) and _p not in sys.path:
        sys.path.append(_p)

import concourse.bass as bass
import concourse.bacc as bacc
import concourse.mybir as mybir
import concourse.tile as tile
from concourse.bass import AP
from concourse.bass_utils import run_bass_kernel_spmd

F32 = mybir.dt.float32
BF16 = mybir.dt.bfloat16
FP8 = mybir.dt.float8e4
I32 = mybir.dt.int32

NPF32 = np.float32
NPBF16 = mybir.dt.np(BF16)
NPFP8 = mybir.dt.np(FP8)

# Problem constants
N, E, D, H, CDIM, L = 50000, 800000, 128, 4, 32, 2
P = 128
NCORES = 8
NEG = 0.2

# dtype knobs
_EDT = os.environ.get("KDT_EDGE", "bf16")
_SDT = os.environ.get("KDT_SEL", "fp8")
EDGE_DT = {"bf16": BF16, "f32": F32}[_EDT]
NP_EDGE = mybir.dt.np(EDGE_DT)
SEL_DT = {"fp8": FP8, "bf16": BF16, "f32": F32}[_SDT]
NP_SEL = mybir.dt.np(SEL_DT)
TRACE = bool(int(os.environ.get("KTRACE", "0")))

LAST_EXEC_NS = []     # per-launch exec_time_ns when tracing
LAST_RESULTS = []     # per-launch BassKernelResults when tracing


def _install_ntff_hook():
    """Provide antenv.axon_hooks (NTFF profiling via the axon PJRT .so)
    when the image lacks it, so run_bass_kernel_spmd(trace=True) works."""
    try:
        import antenv.axon_hooks  # noqa: F401
        return
    except ImportError:
        pass
    import contextlib
    import ctypes
    import types

    try:
        import antenv
    except ImportError:
        return
    so_path = "/opt/axon/libaxon_pjrt.so"
    if not os.path.exists(so_path):
        return
    lib = ctypes.CDLL(so_path)
    if not hasattr(lib, "axon_start_nrt_profile"):
        return
    lib.axon_start_nrt_profile.argtypes = [
        ctypes.POINTER(ctypes.c_int64),
        ctypes.c_size_t,
    ]
    lib.axon_start_nrt_profile.restype = ctypes.c_int64
    lib.axon_stop_nrt_profile.argtypes = [ctypes.c_char_p]
    lib.axon_stop_nrt_profile.restype = ctypes.c_int64

    @contextlib.contextmanager
    def _hook(output_dir, device_ids):
        import jax

        jax.devices()
        if device_ids:
            ids = (ctypes.c_int64 * len(device_ids))(*device_ids)
            rc = lib.axon_start_nrt_profile(ids, len(device_ids))
        else:
            rc = lib.axon_start_nrt_profile(None, 0)
        if rc != 0:
            raise RuntimeError(f"axon_start_nrt_profile rc={rc}")
        try:
            yield
        finally:
            n = lib.axon_stop_nrt_profile(str(output_dir).encode())
            print(f"ntff profile: {n} file(s) -> {output_dir}", file=sys.stderr)

    mod = types.ModuleType("antenv.axon_hooks")
    _state = {"hook": _hook}
    mod.get_axon_ntff_profile_hook = lambda: _state["hook"]
    mod.set_axon_ntff_profile_hook = lambda h: _state.update(hook=h)
    sys.modules["antenv.axon_hooks"] = mod
    antenv.axon_hooks = mod


if TRACE:
    _install_ntff_hook()


def _bcast_last(ap: AP, n: int) -> AP:
    """Append a stride-0 trailing dim of size n to an AP."""
    return AP(ap.tensor, ap.offset, [list(p) for p in ap.ap] + [[0, n]])


def build_layer_nc(cfg, enable_asserts=False):
    """Build the single-layer SPMD program. cfg: nwin, kt (tiles/window)."""
    NWIN, KT = cfg["nwin"], cfg["kt"]
    NLOCP = NWIN * P
    ESLOT = NWIN * KT * P
    assert KT % 2 == 0
    MACROS = []
    j0 = 0
    while j0 < KT:
        wdt = 4 if KT - j0 >= 4 else KT - j0
        MACROS.append((j0, wdt))
        j0 += wdt
    NMAC = len(MACROS)

    nc = bacc.Bacc(
        "TRN2",
        target_bir_lowering=False,
        debug=False,
        enable_asserts=enable_asserts,
        num_devices=cfg.get("ncores", NCORES),
    )

    # ---- inputs ----
    xgT = nc.dram_tensor("xgT", [P, ESLOT], EDGE_DT, kind="ExternalInput").ap()
    xTloc = nc.dram_tensor("xTloc", [P, NLOCP], F32, kind="ExternalInput").ap()
    Wl_b = nc.dram_tensor("Wl_b", [P, P], EDGE_DT, kind="ExternalInput").ap()
    Wr = nc.dram_tensor("Wr", [P, P], F32, kind="ExternalInput").ap()
    w1 = nc.dram_tensor("w1", [P, P], F32, kind="ExternalInput").ap()
    w2 = nc.dram_tensor("w2", [P, P], F32, kind="ExternalInput").ap()
    We_b = nc.dram_tensor("We_b", [CDIM, P], EDGE_DT, kind="ExternalInput").ap()
    brv = nc.dram_tensor("brv", [1, P], F32, kind="ExternalInput").ap()
    ones1 = nc.dram_tensor("ones1", [1, P], F32, kind="ExternalInput").ap()
    attb4 = nc.dram_tensor("attb4", [P, 4 * P], EDGE_DT, kind="ExternalInput").ap()
    i128f = nc.dram_tensor("i128f", [P, P], F32, kind="ExternalInput").ap()
    b1c = nc.dram_tensor("b1c", [P, 1], F32, kind="ExternalInput").ap()
    b2c = nc.dram_tensor("b2c", [P, 1], F32, kind="ExternalInput").ap()
    bgc = nc.dram_tensor("bgc", [P, 1], F32, kind="ExternalInput").ap()
    eaT = nc.dram_tensor("eaT", [CDIM, ESLOT], EDGE_DT, kind="ExternalInput").ap()
    seld = nc.dram_tensor("seld", [NWIN, P, KT * P], SEL_DT, kind="ExternalInput").ap()
    selTd = nc.dram_tensor("selTd", [NWIN, P, KT * P], SEL_DT, kind="ExternalInput").ap()
    xoutT = nc.dram_tensor("xoutT", [P, NLOCP], F32, kind="ExternalOutput").ap()

    AF = mybir.ActivationFunctionType
    OP = mybir.AluOpType

    with tile.TileContext(nc) as tc:
        with (
            tc.tile_pool(name="const", bufs=1) as cpool,
            tc.tile_pool(name="win", bufs=2) as wpool,
            tc.tile_pool(name="edge", bufs=3) as epool,
            tc.tile_pool(name="psZ", bufs=2, space="PSUM") as psZ,
            tc.tile_pool(name="psV", bufs=2, space="PSUM") as psV,
            tc.tile_pool(name="psO", bufs=1, space="PSUM") as psO,
            tc.tile_pool(name="psE", bufs=1, space="PSUM") as psE,
        ):
            # ---- load constants/weights to SBUF ----
            def cload(ap, shape, dt, tag):
                t = cpool.tile(shape, dt, tag=tag)
                nc.sync.dma_start(out=t[:], in_=ap)
                return t

            Wlb_s = cload(Wl_b, [P, P], EDGE_DT, tag="Wlb_s")
            Wr_s = cload(Wr, [P, P], F32, tag="Wr_s")
            w1_s = cload(w1, [P, P], F32, tag="w1_s")
            w2_s = cload(w2, [P, P], F32, tag="w2_s")
            We_s = cload(We_b, [CDIM, P], EDGE_DT, tag="We_s")
            brv_s = cload(brv, [1, P], F32, tag="brv_s")
            ones_s = cload(ones1, [1, P], F32, tag="ones_s")
            attb_s = cload(attb4, [P, 4 * P], EDGE_DT, tag="attb_s")
            i128f_s = cload(i128f, [P, P], F32, tag="i128f_s")
            b1c_s = cload(b1c, [P, 1], F32, tag="b1c_s")
            b2c_s = cload(b2c, [P, 1], F32, tag="b2c_s")
            bgc_s = cload(bgc, [P, 1], F32, tag="bgc_s")

            # ---- edge + node stage, per window ----
            for w in range(NWIN):
                xgT_sb = wpool.tile([P, KT * P], EDGE_DT, tag="xgT")
                nc.sync.dma_start(
                    out=xgT_sb[:], in_=xgT[:, w * KT * P : (w + 1) * KT * P]
                )
                eaT_sb = wpool.tile([CDIM, KT * P], EDGE_DT, tag="ea")
                nc.sync.dma_start(
                    out=eaT_sb[:], in_=eaT[:, w * KT * P : (w + 1) * KT * P]
                )
                sel_sb = wpool.tile([P, KT * P], SEL_DT, tag="sel")
                nc.sync.dma_start(out=sel_sb[:], in_=seld[w])
                selT_sb = wpool.tile([P, KT * P], SEL_DT, tag="selT")
                nc.sync.dma_start(out=selT_sb[:], in_=selTd[w])

                # xr for this window's 128 local nodes (+ bl + br folded in)
                xtl_sb = wpool.tile([P, P], F32, tag="xtl")
                nc.sync.dma_start(out=xtl_sb[:], in_=xTloc[:, w * P : (w + 1) * P])
                xr_ps = psE.tile([P, P], F32, tag="xr")
                nc.tensor.matmul(out=xr_ps[:], lhsT=xtl_sb[:], rhs=Wr_s[:],
                                 start=True, stop=False)
                nc.tensor.matmul(out=xr_ps[:], lhsT=ones_s[:], rhs=brv_s[:],
                                 start=False, stop=True)
                xr_sb = wpool.tile([P, P], EDGE_DT, tag="xrs")
                nc.scalar.activation(xr_sb[:], xr_ps[:], AF.Copy)

                out12 = psO.tile([P, 132], F32, tag="o12")

                for mi, (j0, MW) in enumerate(MACROS):
                    zq = psZ.tile([P, MW * P], F32, tag="zq")
                    vq = psV.tile([P, MW * P], F32, tag="vq")
                    # z = ea@We + xr[dst] + xg@Wl accumulated in psum
                    for u in range(MW):
                        j = j0 + u
                        nc.tensor.matmul(
                            out=zq[:, u * P : (u + 1) * P],
                            lhsT=eaT_sb[:, j * P : (j + 1) * P],
                            rhs=We_s[:],
                            start=(u == 0),
                            stop=False,
                        )
                    for u in range(MW):
                        j = j0 + u
                        nc.tensor.matmul(
                            out=zq[:, u * P : (u + 1) * P],
                            lhsT=selT_sb[:, j * P : (j + 1) * P],
                            rhs=xr_sb[:],
                            start=False,
                            stop=False,
                        )
                    for u in range(MW):
                        j = j0 + u
                        nc.tensor.matmul(
                            out=zq[:, u * P : (u + 1) * P],
                            lhsT=xgT_sb[:, j * P : (j + 1) * P],
                            rhs=Wlb_s[:],
                            start=False,
                            stop=(u == MW - 1),
                        )
                    # value projection xl = xg@Wl (edge-major psum; bl's
                    # softmax-weighted contribution is bl*denom/(denom+eps)
                    # ~= bl, added with the gat bias in the epilogue)
                    for u in range(MW):
                        j = j0 + u
                        nc.tensor.matmul(
                            out=vq[:, u * P : (u + 1) * P],
                            lhsT=xgT_sb[:, j * P : (j + 1) * P],
                            rhs=Wlb_s[:],
                            start=(u == 0),
                            stop=(u == MW - 1),
                        )
                    # leaky = max(z, 0.2z), then alpha = per-head dot(att, m)
                    t02 = epool.tile([P, MW * P], EDGE_DT, tag="t02")
                    nc.scalar.activation(t02[:], zq[:], AF.Copy, scale=NEG)
                    m_sb = epool.tile([P, MW * P], EDGE_DT, tag="m_sb")
                    nc.vector.tensor_tensor(m_sb[:], zq[:], t02[:], op=OP.max)
                    am = epool.tile([P, MW * P], EDGE_DT, tag="am")
                    nc.vector.tensor_tensor(am[:], m_sb[:], attb_s[:, : MW * P],
                                            op=OP.mult)
                    alpha = epool.tile([P, 4 * MW], F32, tag="alpha")
                    nc.vector.tensor_reduce(
                        alpha[:],
                        am[:].rearrange("p (g c) -> p g c", c=CDIM),
                        mybir.AxisListType.X,
                        OP.add,
                    )
                    comb = epool.tile([P, MW * 132], EDGE_DT, tag="comb")
                    comb_v = comb[:].rearrange("p (b f) -> p b f", f=132)
                    nc.scalar.activation(
                        comb_v[:, :, P : P + 4],
                        alpha[:].rearrange("p (b h) -> p b h", h=4),
                        AF.Exp,
                    )
                    nc.vector.tensor_tensor(
                        comb_v[:, :, 0:P].rearrange("p b (h c) -> p b h c", c=CDIM),
                        vq[:].rearrange("p (b h c) -> p b h c", b=MW, c=CDIM),
                        _bcast_last(comb_v[:, :, P : P + 4], CDIM),
                        op=OP.mult,
                    )
                    for u in range(MW):
                        j = j0 + u
                        nc.tensor.matmul(
                            out=out12[:],
                            lhsT=sel_sb[:, j * P : (j + 1) * P],
                            rhs=comb[:, u * 132 : (u + 1) * 132],
                            start=(mi == 0 and u == 0),
                            stop=(mi == len(MACROS) - 1 and u == MW - 1),
                        )

                # ---- window epilogue ----
                de = wpool.tile([P, 4], F32, tag="de")
                nc.vector.tensor_scalar(de[:], out12[:, P : P + 4], 1e-16, None,
                                        OP.add)
                rc = wpool.tile([P, 4], F32, tag="rc")
                nc.vector.reciprocal(rc[:], de[:])
                gat = wpool.tile([P, P], F32, tag="gat")
                for h in range(H):
                    nc.vector.tensor_scalar(
                        gat[:, h * CDIM : (h + 1) * CDIM],
                        out12[:, h * CDIM : (h + 1) * CDIM],
                        rc[:, h : h + 1],
                        None,
                        OP.mult,
                    )
                gatT_ps = psE.tile([P, P], F32, tag="epi")
                nc.tensor.transpose(gatT_ps[:], gat[:], i128f_s[:])
                gTb = wpool.tile([P, P], F32, tag="gTb")
                nc.scalar.activation(gTb[:], gatT_ps[:], AF.Identity, bias=bgc_s[:])
                y1_ps = psE.tile([P, P], F32, tag="epi")
                nc.tensor.matmul(out=y1_ps[:], lhsT=w1_s[:], rhs=gTb[:],
                                 start=True, stop=True)
                y1s = wpool.tile([P, P], F32, tag="y1s")
                nc.scalar.activation(y1s[:], y1_ps[:], AF.Relu, bias=b1c_s[:])
                y2_ps = psE.tile([P, P], F32, tag="epi")
                nc.tensor.matmul(out=y2_ps[:], lhsT=w2_s[:], rhs=y1s[:],
                                 start=True, stop=True)
                xo = wpool.tile([P, P], F32, tag="xo")
                nc.scalar.activation(xo[:], y2_ps[:], AF.Identity, bias=b2c_s[:])
                nc.sync.dma_start(out=xoutT[:, w * P : (w + 1) * P], in_=xo[:])

    nc.compile()
    return nc


# ----------------------------------------------------------------------------
# Host-side preprocessing
# ----------------------------------------------------------------------------

def _preprocess(edge_index, edge_attr, ncores, nloc, nwin):
    """Route edges per core into gather-ready slot arrays.

    Node->window assignment is degree-balanced (LPT) so every window has a
    near-equal edge count, minimizing the uniform KT tile budget. Slot s of
    window w: tile j = s // 128, edge lane q = s % 128. Returns per-core
    dicts with src_slot (for the per-layer host halo gather), one-hot
    Sel/SelT, eaT, and the node permutation, plus kt.
    """
    src = np.ascontiguousarray(edge_index[0]).astype(np.int64)
    dst = np.ascontiguousarray(edge_index[1]).astype(np.int64)
    n = nloc * ncores
    ea = np.ascontiguousarray(edge_attr, dtype=np.float32)

    deg = np.bincount(dst, minlength=n).astype(np.float32)
    order = np.argsort(dst, kind="stable")
    dst_s = dst[order]
    src_s = src[order]
    ea_s = ea[order]
    cs = np.concatenate(
        [np.zeros((1, ea.shape[1]), np.float64), np.cumsum(ea_s, 0, dtype=np.float64)]
    )
    starts = np.searchsorted(dst_s, np.arange(n))
    ends = np.searchsorted(dst_s, np.arange(n) + 1)
    loop_attr = ((cs[ends] - cs[starts]) / np.maximum(deg, 1.0)[:, None]).astype(
        np.float32
    )

    import heapq

    cores = []
    maxcnt = 0
    for c in range(ncores):
        base = c * nloc
        # edges of this core (dst-local), self-loops appended
        lo, hi = starts[base], ends[base + nloc - 1]
        s2 = np.concatenate([src_s[lo:hi], np.arange(base, base + nloc)])
        dl = np.concatenate([dst_s[lo:hi], np.arange(base, base + nloc)]) - base
        e2 = np.concatenate([ea_s[lo:hi], loop_attr[base : base + nloc]], 0)

        # LPT balance: assign local nodes (weight = deg+1) to nwin windows
        w_of = np.empty(nloc, np.int64)
        pos_of = np.empty(nloc, np.int64)
        wdeg = (deg[base : base + nloc] + 1.0).astype(np.int64)
        heap = [(0, w, 0) for w in range(nwin)]  # (total, window, fill)
        heapq.heapify(heap)
        for node in np.argsort(-wdeg):
            tot, w, fill = heapq.heappop(heap)
            w_of[node] = w
            pos_of[node] = fill
            fill += 1
            tot += int(wdeg[node])
            if fill < P:
                heapq.heappush(heap, (tot, w, fill))
            else:
                heapq.heappush(heap, (1 << 60, w, fill))
        we = w_of[dl]
        pe_ = pos_of[dl]
        o = np.argsort(we, kind="stable")
        s2, e2, we, pe_ = s2[o], e2[o], we[o], pe_[o]
        wstart = np.searchsorted(we, np.arange(nwin))
        wend = np.searchsorted(we, np.arange(nwin) + 1)
        cnts = wend - wstart
        maxcnt = max(maxcnt, int(cnts.max()))
        # node permutation: slot w*128+pos holds local node id
        nl_flat = np.zeros(nwin * P, np.int64)
        nl_flat[w_of * P + pos_of] = np.arange(nloc)
        used = np.zeros(nwin * P, bool)
        used[w_of * P + pos_of] = True
        cores.append((s2, e2, pe_, wstart, cnts, nl_flat, used))

    kt = -(-maxcnt // P)
    if kt % 2:
        kt += 1
    S = kt * P

    data = []
    for (s2, e2, pe_, wstart, cnts, nl_flat, used) in cores:
        nslot = nwin * S
        src_slot = np.zeros(nslot, np.int64)
        dstw_slot = np.full(nslot, -1, np.int64)
        ea_slot = np.zeros((nslot, CDIM), np.float32)
        idx = np.concatenate([np.arange(cnts[w]) + w * S for w in range(nwin)])
        src_slot[idx] = s2
        dstw_slot[idx] = pe_
        ea_slot[idx] = e2

        dw = dstw_slot.reshape(nwin, kt, P)  # [w, j, q]
        sel = (dw[:, :, :, None] == np.arange(P)[None, None, None, :])
        sel = sel.transpose(0, 2, 1, 3).reshape(nwin, P, kt * P).astype(NP_SEL)
        selT = (dw[:, :, None, :] == np.arange(P)[None, None, :, None])
        selT = selT.transpose(0, 2, 1, 3).reshape(nwin, P, kt * P).astype(NP_SEL)
        eaT = np.ascontiguousarray(ea_slot.T).astype(NP_EDGE)
        data.append(dict(src_slot=src_slot, seld=sel, selTd=selT, eaT=eaT,
                         nl_flat=nl_flat, used=used))
    return data, kt


def _layer_weight_maps(inputs, layer, att):
    """Shared (same for all cores) weight/const arrays for one layer."""
    i = layer
    attf = att[i].reshape(-1).astype(np.float32)  # [128]
    m = dict(
        Wl_b=np.ascontiguousarray(inputs["Wl"][i]).astype(NP_EDGE),
        Wr=np.ascontiguousarray(inputs["Wr"][i]).astype(NPF32),
        w1=np.ascontiguousarray(inputs["w1"][i]).astype(NPF32),
        w2=np.ascontiguousarray(inputs["w2"][i]).astype(NPF32),
        We_b=np.ascontiguousarray(inputs["We"][i]).astype(NP_EDGE),
        # bl + br both ride the per-dst xr one-hot expansion into z
        brv=(np.asarray(inputs["br"][i]) + np.asarray(inputs["bl"][i]))
        .reshape(1, P)
        .astype(NPF32),
        ones1=np.ones((1, P), NPF32),
        attb4=np.tile(attf[None, :], (P, 4)).astype(NP_EDGE),
        i128f=np.eye(P, dtype=NPF32),
        b1c=np.asarray(inputs["b1"][i]).reshape(P, 1).astype(NPF32),
        b2c=np.asarray(inputs["b2"][i]).reshape(P, 1).astype(NPF32),
        # gat bias + bl (bl rides the normalized softmax weights, sum ~= 1)
        bgc=(np.asarray(inputs["bias"][i]) + np.asarray(inputs["bl"][i]))
        .reshape(P, 1)
        .astype(NPF32),
    )
    return m


_NC_CACHE = {}


def kernel(**inputs):
    nodes = np.asarray(inputs["nodes"], dtype=np.float32)
    edge_index = np.asarray(inputs["edge_index"])
    edge_attr = np.asarray(inputs["edge_attr"], dtype=np.float32)

    n, d = nodes.shape
    assert (n, d) == (N, D)
    nloc = n // NCORES
    nwin = -(-nloc // P)
    nlocp = nwin * P

    data, kt = _preprocess(edge_index, edge_attr, NCORES, nloc, nwin)

    key = (nwin, kt, NCORES)
    if key not in _NC_CACHE:
        _NC_CACHE[key] = build_layer_nc(dict(nwin=nwin, kt=kt, ncores=NCORES))
    nc = _NC_CACHE[key]

    x_curr = np.ascontiguousarray(nodes.T)  # [128, n] f32

    for layer in range(L):
        wmap = _layer_weight_maps(inputs, layer, np.asarray(inputs["att"]))
        xce = x_curr.astype(NP_EDGE)
        in_maps = []
        for c in range(NCORES):
            base = c * nloc
            xTloc = x_curr[:, base + data[c]["nl_flat"]].copy()
            xTloc[:, ~data[c]["used"]] = 0.0
            m = dict(wmap)
            m["xgT"] = np.ascontiguousarray(xce[:, data[c]["src_slot"]])
            m["xTloc"] = xTloc
            m["seld"] = data[c]["seld"]
            m["selTd"] = data[c]["selTd"]
            m["eaT"] = data[c]["eaT"]
            in_maps.append(m)
        res = run_bass_kernel_spmd(
            nc, in_maps, core_ids=list(range(NCORES)), trace=TRACE
        )
        if res.exec_time_ns is not None:
            LAST_EXEC_NS.append(res.exec_time_ns)
        if TRACE:
            LAST_RESULTS.append(res)
        outs = res.results
        x_next = np.zeros((P, n), NPF32)
        for c in range(NCORES):
            xo = outs[c]["xoutT"]
            u = data[c]["used"]
            x_next[:, c * nloc + data[c]["nl_flat"][u]] = xo[:, u]
        x_curr = x_next

    return np.ascontiguousarray(x_curr.T.astype(np.float32))
